# revision 1
# baseline (speedup 1.0000x reference)
"""Trainium2 Bass kernel for a bidirectional GRU language model head.

Model (see problem reference): tokens x[T=64, B=64] -> embedding[32000, 32]
-> forward GRU (H=8, scalar z/r gates) + backward GRU -> concat [T,B,16]
-> logits = h @ Wout[16, 32000] + bout -> log_softmax over vocab.

Output is [64, 64, 32000] f32 = 512 MB, so the kernel is memory bound on
the output write (~64 MB/core across 8 cores, ~360 GB/s HBM per core).

Sharding: data-parallel over batch. Core c gets batch columns [8c, 8c+8);
it runs the full T=64 recurrence for its 8 sequences and the full-vocab
projection + log-softmax for its 512 tokens. No collectives.

Compute-engine SBUF access patterns must start at partition 0/32/64/96,
so the two GRU directions live in a "spread" layout: forward state at
partitions 0:8, backward at 32:40, with zero padding baked into the
weights (junk lanes multiply against zero weight columns).

Device-side plan per core:
  1. Gather embeddings for the 512 tokens with indirect DMA, transpose to
     [32, tok] with the PE, and precompute all input-side gate terms
     P20 = We_all.T @ [enc; 1] in one matmul (biases folded in).
  2. Run both GRU directions together in transposed [H, B] layout, 63
     dependent steps: PE does the tiny gate matmuls, ACT sigmoid/tanh,
     DVE elementwise + stream_shuffles that broadcast the scalar z/r
     gates across partitions. Pre-update states stream into HT tiles.
  3. Projection per 128-token tile: logits = HTb.T @ Wout_aug (K=65,
     bf16, bias folded via ones lanes). Pass 1 computes sum(exp(logits))
     with ACT exp+accumulate straight out of PSUM (no max-shift needed:
     |logits| <= 4.25). Pass 2 recomputes the matmul and writes
     logits - logsumexp into a staging buffer (ACT/DVE split), DMA'd out
     in 4 MB pieces.
"""

import numpy as np
import ml_dtypes

VOCAB, HID, EMB = 32000, 8, 32
SEQ, BATCH = 64, 64
NCORES = 8
BS = BATCH // NCORES          # batch columns per core
TOK = SEQ * BS                # tokens per core
NCHUNK = 500                  # vocab columns per matmul (PSUM bank = 512 f32)

_module_cache = {}


def _build_module(vocab=VOCAB, act_sub_every=16, stage_chunks=16, proj_order=(1, 2, 0, 3), reps=1, upto_scan=False, serialize_reps=False):
    import concourse.bass as bass
    import concourse.bacc as bacc
    import concourse.mybir as mybir
    import concourse.tile as tile
    from concourse.masks import make_identity

    dt = mybir.dt
    AF = mybir.ActivationFunctionType

    nch = vocab // NCHUNK
    assert nch * NCHUNK == vocab
    stage_chunks = min(stage_chunks, nch)
    assert nch % stage_chunks == 0

    nc = bacc.Bacc("TRN2", target_bir_lowering=False, debug=False)

    x_d = nc.dram_tensor("x", [SEQ, BS], dt.int32, kind="ExternalInput")
    emb_d = nc.dram_tensor("emb", [vocab, EMB], dt.float32, kind="ExternalInput")
    wea_d = nc.dram_tensor("wea", [EMB + 1, 104], dt.float32, kind="ExternalInput")
    wzr_d = nc.dram_tensor("wzr", [98, 128], dt.float32, kind="ExternalInput")
    whh_d = nc.dram_tensor("whh", [64, 64], dt.float32, kind="ExternalInput")
    wout_d = nc.dram_tensor("wout", [65, vocab], dt.bfloat16, kind="ExternalInput")
    out_d = nc.dram_tensor("out", [TOK, vocab], dt.float32, kind="ExternalOutput")

    NT = TOK // 128  # 128-token projection tiles (4)

    with tile.TileContext(nc) as tc:
        with (
            tc.tile_pool(name="const", bufs=1) as cpool,
            tc.tile_pool(name="scan", bufs=2) as spool,
            tc.tile_pool(name="scan1", bufs=1) as s1pool,
            tc.tile_pool(name="stage", bufs=int(__import__("os").environ.get("STG_BUFS", "2"))) as stgp,
            tc.tile_pool(name="small", bufs=2) as smp,
        ):
            # ---- constants / inputs to SBUF ----
            wout_sb = cpool.tile([65, vocab], dt.bfloat16)
            nc.sync.dma_start(wout_sb[:], wout_d[:])
            wea_sb = cpool.tile([EMB + 1, 104], dt.float32)
            nc.sync.dma_start(wea_sb[:], wea_d[:])
            wzr_sb = cpool.tile([98, 128], dt.float32)
            nc.sync.dma_start(wzr_sb[:], wzr_d[:])
            whh_sb = cpool.tile([64, 64], dt.float32)
            nc.sync.dma_start(whh_sb[:], whh_d[:])
            ident_sb = cpool.tile([128, 128], dt.float32)
            make_identity(nc, ident_sb[:])
            idx_sb = cpool.tile([128, NT], dt.int32)
            # token g*128+p lives at x[(g*16 + p//8), p%8]
            nc.sync.dma_start(idx_sb[:], x_d.ap().rearrange("(g q) b -> (q b) g", g=NT))

            encT = cpool.tile([EMB + 1, TOK], dt.float32)
            nc.vector.memset(encT[EMB : EMB + 1, :], 1.0)
            # P20 rows (quadrant-aligned): 0:2 = z1,r1; 32:34 = z2,r2;
            # 64:72 = h1e; 96:104 = h2e.  Biases folded via encT ones row.
            P20 = cpool.tile([104, TOK], dt.float32)
            # P20EH [64, TOK]: rows 0:8 = h1e in token order; rows 32:40 = h2e
            # in REVERSED block order (block j holds e-terms of t = 63-j), so a
            # single [64]-row add serves both scan directions each step.
            P20EH = cpool.tile([64, TOK], dt.float32)
            HT = [cpool.tile([40, 128], dt.float32, name=f"HT{m}", tag=f"HT{m}")
                  for m in range(NT)]
            HTb = [cpool.tile([65, 128], dt.bfloat16, name=f"HTb{m}", tag=f"HTb{m}")
                   for m in range(NT)]
            for m in range(NT):
                # 1.0 everywhere: row 64 is the bias ones-row; unused lanes
                # (8:32, 40:64) hit zero rows of wout so any finite value works.
                nc.vector.memset(HTb[m][:], 1.0)

            for rep in range(reps):
                if serialize_reps and rep > 0:
                    # force rep r to start only after rep r-1's output DMA:
                    # read back a slab of out_d, zero it, and fold it into the
                    # gather indices so the whole body chains behind it.
                    dscr = smp.tile([128, NT], dt.float32, tag="dscr")
                    nc.sync.dma_start(dscr[:], out_d[0:128, 0:NT])
                    nc.vector.tensor_scalar_mul(dscr[:], dscr[:], 0.0)
                    dzero = smp.tile([128, NT], dt.int32, tag="dzero")
                    nc.vector.tensor_copy(dzero[:], dscr[:])
                    idx_use = smp.tile([128, NT], dt.int32, tag="idxuse")
                    nc.vector.tensor_add(idx_use[:], idx_sb[:], dzero[:])
                else:
                    idx_use = idx_sb
                # ---- phase 1: embedding gather -> encT -> P20 ----
                with (
                    tc.tile_pool(name="gath", bufs=2) as gpool,
                    tc.tile_pool(name="pst", bufs=1, space="PSUM") as pstp,
                ):
                    for g in range(NT):
                        encg = gpool.tile([128, EMB], dt.float32, tag="encg")
                        nc.gpsimd.indirect_dma_start(
                            out=encg[:],
                            out_offset=None,
                            in_=emb_d.ap(),
                            in_offset=bass.IndirectOffsetOnAxis(ap=idx_use[:, g : g + 1], axis=0),
                        )
                        pst = pstp.tile([EMB, 128], dt.float32, tag="pst")
                        nc.tensor.transpose(out=pst[:], in_=encg[:], identity=ident_sb[:])
                        nc.vector.tensor_copy(encT[0:EMB, g * 128 : (g + 1) * 128], pst[:])
                    p20ps = pstp.tile([104, TOK], dt.float32, tag="p20")
                    nc.tensor.matmul(p20ps[:], lhsT=wea_sb[:], rhs=encT[:], start=True, stop=True)
                    nc.vector.tensor_copy(P20[:], p20ps[:])
                    nc.vector.memset(P20EH[:], 0.0)
                    nc.vector.tensor_copy(P20EH[0:8, :], p20ps[64:72, :])

                if True:

                    # ---- phase 2: the two GRU scans, interleaved, 63 steps ----
                    # state S [98, BS]: rows 0:8 forward h, rows 32:40 backward h,
                    # rows 64:66 = fwd ezr (z1,r1 input-side terms for this step),
                    # rows 96:98 = bwd ezr.  The zr matmul contracts over all 98
                    # rows: selector rows 64:98 of wzr add the e-terms, avoiding a
                    # multi-matmul PSUM accumulation group (which hangs on HW).
                    zrpsp = tc.alloc_tile_pool(name="zrps", bufs=1, space="PSUM")
                    gpsp = tc.alloc_tile_pool(name="gps", bufs=1, space="PSUM")
                    lpsp = tc.alloc_tile_pool(name="lps", bufs=2, space="PSUM")
                    epsp = tc.alloc_tile_pool(name="eps", bufs=1, space="PSUM")

                    zr4 = s1pool.tile([128, BS], dt.float32)
                    # reversed-order bwd e-term copies trickle in during the
                    # scan: block j is only needed at step j (subtile deps)
                    for j in range(SEQ):
                        nc.vector.tensor_copy(
                            P20EH[32:40, j * BS : (j + 1) * BS],
                            P20[96:104, (SEQ - 1 - j) * BS : (SEQ - j) * BS])
                    S = spool.tile([98, BS], dt.float32, tag="S")
                    nc.vector.memset(S[0:64, :], 0.0)
                    # P20 rows 2:32 are zero, so this fills 64:96 with [ezr_f; 0...]
                    nc.vector.tensor_copy(S[64:96, :], P20[0:32, 0:BS])
                    nc.vector.tensor_copy(S[96:98, :], P20[32:34, (SEQ - 1) * BS : SEQ * BS])
                    nc.vector.memset(HT[0][0:8, 0:BS], 0.0)              # fwd state 0 @ block 0
                    nc.vector.memset(HT[NT - 1][32:40, 128 - BS : 128], 0.0)  # bwd state 0 @ block 63
                    # one mask: every output row of quadrant q copies input
                    # row 32q (z1/z2/r1/r2 live at rows 0/32/64/96 of zr4)
                    mask_z = [0] * 32

                    for s in range(SEQ - 1):
                        fcol = s * BS               # fwd step s consumes e_t, t = s
                        bcol = (SEQ - 1 - s) * BS   # bwd step s consumes e_t, t = 63 - s
                        # zr gates spread over quadrants: rows 0=z1, 32=z2,
                        # 64=r1, 96=r2 (e-terms included via selector rows),
                        # so ONE stream_shuffle broadcasts z to rows 0:64 and
                        # r to rows 64:128.
                        zrps = zrpsp.tile([128, BS], dt.float32, tag="zr")
                        nc.tensor.matmul(zrps[:], lhsT=wzr_sb[:], rhs=S[:], start=True, stop=True)
                        gps = gpsp.tile([64, BS], dt.float32, tag="g")
                        nc.tensor.matmul(gps[:], lhsT=whh_sb[:], rhs=S[0:64, :], start=True, stop=True)
                        nc.scalar.activation(out=zr4[:], in_=zrps[:], func=AF.Sigmoid)
                        bc = spool.tile([128, BS], dt.float32, tag="bc")
                        nc.vector.stream_shuffle(out=bc[:], in_=zr4[:], mask=mask_z)
                        # z-path (off critical path): v = h - z*h
                        u = spool.tile([64, BS], dt.float32, tag="u")
                        nc.vector.tensor_mul(u[:], S[0:64, :], bc[0:64, :])
                        v = spool.tile([64, BS], dt.float32, tag="v")
                        nc.vector.tensor_sub(v[:], S[0:64, :], u[:])
                        # r-path, in place in PSUM: cand = tanh(r * (Whh.T h) + eh)
                        nc.vector.tensor_mul(gps[:], gps[:], bc[64:128, :])
                        nc.vector.tensor_add(gps[:], gps[:], P20EH[:, fcol : fcol + BS])
                        cand = spool.tile([64, BS], dt.float32, tag="cand")
                        nc.scalar.activation(out=cand[:], in_=gps[:], func=AF.Tanh)
                        w = spool.tile([64, BS], dt.float32, tag="w")
                        nc.vector.tensor_mul(w[:], cand[:], bc[0:64, :])
                        S2 = spool.tile([98, BS], dt.float32, tag="S")
                        nc.vector.tensor_add(S2[0:64, :], v[:], w[:])
                        # load next step's input-side zr terms (static data, off
                        # the critical path; P20 rows 2:32 are zero)
                        nc.vector.tensor_copy(S2[64:96, :], P20[0:32, fcol + BS : fcol + 2 * BS])
                        nc.vector.tensor_copy(S2[96:98, :], P20[32:34, bcol - BS : bcol])
                        # store pre-update states: fwd block s+1, bwd block 62-s
                        fb = s + 1
                        bb = SEQ - 2 - s
                        nc.gpsimd.tensor_copy(HT[fb // 16][0:8, (fb % 16) * BS : (fb % 16) * BS + BS],
                                              S2[0:8, :])
                        nc.gpsimd.tensor_copy(HT[bb // 16][32:40, (bb % 16) * BS : (bb % 16) * BS + BS],
                                              S2[32:40, :])
                        S = S2

                    if upto_scan:
                        for m in range(NT):
                            nc.sync.dma_start(out_d[m * 40 : m * 40 + 40, 0:128], HT[m][:])
                    else:
                        # ---- phase 3: projection + log_softmax per 128-token tile ----
                        for m in proj_order:
                            nc.vector.tensor_copy(HTb[m][0:8, :], HT[m][0:8, :])
                            nc.vector.tensor_copy(HTb[m][32:40, :], HT[m][32:40, :])
                            sums = smp.tile([128, nch // 2], dt.float32, tag="sums")
                            for j2 in range(nch // 2):
                                lps = lpsp.tile([128, 2, 512], dt.float32, tag="l")
                                for h in range(2):
                                    j = 2 * j2 + h
                                    nc.tensor.matmul(lps[:, h, 0:NCHUNK],
                                                     lhsT=HTb[m][:],
                                                     rhs=wout_sb[:, j * NCHUNK : (j + 1) * NCHUNK],
                                                     start=True, stop=True)
                                eps = epsp.tile([128, 2, 512], dt.float32, tag="e")
                                nc.scalar.activation(out=eps[:, :, 0:NCHUNK], in_=lps[:, :, 0:NCHUNK],
                                                     func=AF.Exp,
                                                     accum_out=sums[:, j2 : j2 + 1])
                            nlz = smp.tile([128, 2], dt.float32, tag="nlz")
                            nc.vector.reduce_sum(out=nlz[:, 0:1], in_=sums[:, 0 : nch // 2], axis=mybir.AxisListType.X)
                            nc.scalar.activation(out=nlz[:, 1:2], in_=nlz[:, 0:1], func=AF.Ln)
                            nc.vector.tensor_scalar_mul(nlz[:, 0:1], nlz[:, 1:2], -1.0)
                            for q in range(nch // stage_chunks):
                                stg = stgp.tile([128, stage_chunks * NCHUNK], dt.float32, tag="stg")
                                for jj2 in range(stage_chunks // 2):
                                    lps = lpsp.tile([128, 2, 512], dt.float32, tag="l")
                                    for h in range(2):
                                        j = q * stage_chunks + 2 * jj2 + h
                                        nc.tensor.matmul(lps[:, h, 0:NCHUNK],
                                                         lhsT=HTb[m][:],
                                                         rhs=wout_sb[:, j * NCHUNK : (j + 1) * NCHUNK],
                                                         start=True, stop=True)
                                    dst = stg[:, 2 * jj2 * NCHUNK : (2 * jj2 + 2) * NCHUNK]
                                    dst = dst.rearrange("p (two c) -> p two c", two=2)
                                    if (q * (stage_chunks // 2) + jj2) % act_sub_every == 0:
                                        nc.scalar.add(dst, lps[:, :, 0:NCHUNK], nlz[:, 0:1])
                                    else:
                                        nc.vector.tensor_scalar_add(dst, lps[:, :, 0:NCHUNK], nlz[:, 0:1])
                                import os as _os2
                                if _os2.environ.get("DMA_DIV", "1") == "1":
                                    _eng = nc.sync
                                    if _os2.environ.get("DMA_ENG", "sync") == "mix":
                                        _eng = (nc.sync, nc.gpsimd)[(m * 8 + q) % 2]
                                    elif _os2.environ.get("DMA_ENG") == "gps":
                                        _eng = nc.gpsimd
                                    _eng.dma_start(
                                        out_d[m * 128 : (m + 1) * 128,
                                              q * stage_chunks * NCHUNK : (q + 1) * stage_chunks * NCHUNK],
                                        stg[:],
                                    )
                                else:
                                    dv = int(_os2.environ["DMA_DIV"])
                                    nc.sync.dma_start(
                                        out_d[m * 128 : (m + 1) * 128,
                                              q * stage_chunks * NCHUNK : q * stage_chunks * NCHUNK
                                              + stage_chunks * NCHUNK // dv],
                                        stg[:, 0 : stage_chunks * NCHUNK // dv],
                                    )
                    for p in (epsp, lpsp, gpsp, zrpsp):
                        p.release()

    nc.compile()
    return nc


def _prep_weights(embeddings, Wz1, bz1, Wr1, br1, Wh1, bh1, Wz2, bz2, Wr2, br2, Wh2, bh2,
                  Wout, bout):
    f32 = np.float32
    emb = np.ascontiguousarray(np.asarray(embeddings, dtype=f32))
    vocab = emb.shape[0]

    Wz1, Wr1, Wh1 = (np.asarray(a, dtype=f32) for a in (Wz1, Wr1, Wh1))
    Wz2, Wr2, Wh2 = (np.asarray(a, dtype=f32) for a in (Wz2, Wr2, Wh2))

    # We_all [33, 104]: embedding-side weights for all gates, bias row folded
    # in, columns already in the quadrant-aligned P20 row layout:
    # 0=z1, 1=r1, 32=z2, 33=r2, 64:72=h1, 96:104=h2.  cat = [h, e].
    wea = np.zeros((EMB + 1, 104), dtype=f32)
    wea[:EMB, 0] = Wz1[HID:, 0]
    wea[:EMB, 1] = Wr1[HID:, 0]
    wea[:EMB, 32] = Wz2[HID:, 0]
    wea[:EMB, 33] = Wr2[HID:, 0]
    wea[:EMB, 64:72] = Wh1[HID:, :]
    wea[:EMB, 96:104] = Wh2[HID:, :]
    wea[EMB, 0] = np.asarray(bz1)[0]
    wea[EMB, 1] = np.asarray(br1)[0]
    wea[EMB, 32] = np.asarray(bz2)[0]
    wea[EMB, 33] = np.asarray(br2)[0]
    wea[EMB, 64:72] = np.asarray(bh1)
    wea[EMB, 96:104] = np.asarray(bh2)

    # Wzr spread [98, 128]: hidden-side z/r weights plus selector rows that
    # pass through the precomputed input-side terms carried in S rows 64:98.
    # Output rows (one per quadrant so one stream_shuffle broadcasts all
    # four gates): 0=z1, 32=z2, 64=r1, 96=r2.  State rows: fwd 0:8, bwd 32:40.
    wzr = np.zeros((98, 128), dtype=f32)
    wzr[0:HID, 0] = Wz1[:HID, 0]
    wzr[32 : 32 + HID, 32] = Wz2[:HID, 0]
    wzr[0:HID, 64] = Wr1[:HID, 0]
    wzr[32 : 32 + HID, 96] = Wr2[:HID, 0]
    wzr[64, 0] = 1.0   # ez1
    wzr[96, 32] = 1.0  # ez2
    wzr[65, 64] = 1.0  # er1
    wzr[97, 96] = 1.0  # er2

    # Whh spread [64, 64]: block "diag" hidden-side candidate weights.
    whh = np.zeros((64, 64), dtype=f32)
    whh[0:HID, 0:HID] = Wh1[:HID, :]
    whh[32 : 32 + HID, 32 : 32 + HID] = Wh2[:HID, :]

    # Wout_aug [65, vocab] bf16: rows 0:8 fwd-h weights, 32:40 bwd-h
    # weights, 64 = bout; all other rows zero (matching HTb junk lanes).
    Wout = np.asarray(Wout, dtype=f32)
    wout_aug = np.zeros((65, vocab), dtype=f32)
    wout_aug[0:HID, :] = Wout[0:HID, :]
    wout_aug[32 : 32 + HID, :] = Wout[HID:, :]
    wout_aug[64, :] = np.asarray(bout, dtype=f32)
    wout_aug = wout_aug.astype(ml_dtypes.bfloat16)

    return dict(emb=emb, wea=wea, wzr=wzr, whh=whh, wout=wout_aug,
                vocab=vocab)


def run(inputs, trace=False):
    from concourse.bass_utils import run_bass_kernel_spmd

    w = _prep_weights(
        inputs["embeddings"],
        inputs["Wz1"], inputs["bz1"], inputs["Wr1"], inputs["br1"],
        inputs["Wh1"], inputs["bh1"],
        inputs["Wz2"], inputs["bz2"], inputs["Wr2"], inputs["br2"],
        inputs["Wh2"], inputs["bh2"],
        inputs["Wout"], inputs["bout"],
    )
    vocab = w.pop("vocab")
    x = np.ascontiguousarray(np.asarray(inputs["x"], dtype=np.int32))
    assert x.shape == (SEQ, BATCH)

    key = ("module", vocab)
    if key not in _module_cache:
        _module_cache[key] = _build_module(vocab=vocab)
    nc = _module_cache[key]

    in_maps = []
    for c in range(NCORES):
        m = dict(w)
        m["x"] = np.ascontiguousarray(x[:, c * BS : (c + 1) * BS])
        in_maps.append(m)

    res = run_bass_kernel_spmd(nc, in_maps, core_ids=list(range(NCORES)), trace=trace)
    shards = [res.results[c]["out"].reshape(SEQ, BS, vocab) for c in range(NCORES)]
    out = np.concatenate(shards, axis=1)
    return out, res


def kernel(**inputs):
    out, _ = run(inputs)
    return out



# revision 2
# speedup vs baseline: 1.0807x; 1.0807x over previous
"""Trainium2 Bass kernel for a bidirectional GRU language model head.

Model (see problem reference): tokens x[T=64, B=64] -> embedding[32000, 32]
-> forward GRU (H=8, scalar z/r gates) + backward GRU -> concat [T,B,16]
-> logits = h @ Wout[16, 32000] + bout -> log_softmax over vocab.

Output is [64, 64, 32000] f32 = 512 MB. Sharding: data-parallel over batch;
core c gets batch columns [8c, 8c+8), runs the full T=64 recurrence for its
8 sequences and the full-vocab projection + log-softmax for its 512 tokens.
No collectives. The hard wall is the per-core HBM write of 64 MB (~360 GB/s),
so the projection is organized to start the output DMA as early as possible
and keep it continuously fed.

Device-side plan per core:
  1. Gather embeddings for the 512 tokens with indirect DMA, transpose to
     [32, tok] with the PE, and precompute all input-side gate terms
     P20 = We_all.T @ [enc; 1] in one matmul (biases folded in).
  2. Run both GRU directions together in transposed [H, B] layout, 63
     dependent steps. All gate nonlinearities are TANH ONLY (sigmoid is
     computed as z = (1+tanh(a/2))/2 with the 1/2 folded into the weights)
     so the scan and the projection (exp/ln) never thrash the ACT engine's
     function-table sets. The z/r gate matmul uses column-replicated weights
     so its PSUM output is already broadcast across partitions (no
     stream_shuffle), and the candidate matmul is augmented with selector
     rows that add the input-side candidate terms (no per-step row add).
  3. Projection per 128-token tile: logits = HTb.T @ Wout_aug (K=65, bf16,
     bias folded via a ones lane). Pass 1 computes sum(exp(logits)) with ACT
     exp+accumulate straight out of PSUM (no max-shift needed: |logits| <=
     4.25). Pass 2 recomputes the matmul and writes logits - logsumexp into
     a staging buffer, DMA'd out in 4 MB pieces. Pass 1 of tile m+1 is
     interleaved at group granularity with pass 2 of tile m so ACT (exp),
     DVE (subtract-drain), PE (matmuls, kept continuously busy so it ramps
     to its 2.4 GHz p-state) and the output DMA all run concurrently.
"""

import numpy as np
import ml_dtypes

VOCAB, HID, EMB = 32000, 8, 32
SEQ, BATCH = 64, 64
NCORES = 8
BS = BATCH // NCORES          # batch columns per core
TOK = SEQ * BS                # tokens per core
NCHUNK = 500                  # vocab columns per matmul (PSUM bank = 512 f32)

_module_cache = {}


def _build_module(vocab=VOCAB):
    import concourse.bass as bass
    import concourse.bacc as bacc
    import concourse.mybir as mybir
    import concourse.tile as tile
    from concourse.masks import make_identity

    dt = mybir.dt
    AF = mybir.ActivationFunctionType
    ALU = mybir.AluOpType

    nch = vocab // NCHUNK          # 64 chunks of 500 per 128-token tile
    assert nch * NCHUNK == vocab
    NG = nch // 2                  # 32 groups of 2 chunks (1000 cols) each
    PIECE_G = 8                    # groups per staged DMA piece (8000 cols, 4 MB)
    assert NG % PIECE_G == 0

    nc = bacc.Bacc("TRN2", target_bir_lowering=False, debug=False)

    x_d = nc.dram_tensor("x", [SEQ, BS], dt.int32, kind="ExternalInput")
    emb_d = nc.dram_tensor("emb", [vocab, EMB], dt.float32, kind="ExternalInput")
    wea_d = nc.dram_tensor("wea", [EMB + 1, 42], dt.float32, kind="ExternalInput")
    wzr_d = nc.dram_tensor("wzr", [106, 128], dt.float32, kind="ExternalInput")
    whh_d = nc.dram_tensor("whh", [106, 128], dt.float32, kind="ExternalInput")
    wout_d = nc.dram_tensor("wout", [65, vocab], dt.bfloat16, kind="ExternalInput")
    out_d = nc.dram_tensor("out", [TOK, vocab], dt.float32, kind="ExternalOutput")

    NT = TOK // 128  # 128-token projection tiles (4)

    with tile.TileContext(nc) as tc:
        with (
            tc.tile_pool(name="const", bufs=1) as cpool,
            tc.tile_pool(name="scan", bufs=2) as spool,
            tc.tile_pool(name="stage", bufs=3) as stgp,
            tc.tile_pool(name="small", bufs=2) as smp,
            tc.tile_pool(name="expsc", bufs=2) as escp,
        ):
            # ---- constants / inputs to SBUF ----
            wout_sb = cpool.tile([65, vocab], dt.bfloat16)
            nc.sync.dma_start(wout_sb[:], wout_d[:])
            wea_sb = cpool.tile([EMB + 1, 42], dt.float32)
            nc.sync.dma_start(wea_sb[:], wea_d[:])
            wzr_sb = cpool.tile([106, 128], dt.float32)
            nc.sync.dma_start(wzr_sb[:], wzr_d[:])
            whh_sb = cpool.tile([106, 128], dt.float32)
            nc.sync.dma_start(whh_sb[:], whh_d[:])
            ident_sb = cpool.tile([128, 128], dt.float32)
            make_identity(nc, ident_sb[:])
            idx_sb = cpool.tile([128, NT], dt.int32)
            # token g*128+p lives at x[(g*16 + p//8), p%8]
            nc.sync.dma_start(idx_sb[:], x_d.ap().rearrange("(g q) b -> (q b) g", g=NT))

            encT = cpool.tile([EMB + 1, TOK], dt.float32)
            nc.vector.memset(encT[EMB : EMB + 1, :], 1.0)
            # P20 rows: 0=ez1, 1=er1, 2:10=eh1 (fwd, token order); 10:32 zero;
            # 32=ez2, 33=er2, 34:42=eh2 (bwd).  Biases folded via encT ones row.
            P20 = cpool.tile([42, TOK], dt.float32)
            HT = [cpool.tile([40, 128], dt.float32, name=f"HT{m}", tag=f"HT{m}")
                  for m in range(NT)]
            HTb = [cpool.tile([65, 128], dt.bfloat16, name=f"HTb{m}", tag=f"HTb{m}")
                   for m in range(NT)]
            for m in range(NT):
                # 1.0 everywhere: row 64 is the bias ones-row; unused lanes
                # (8:32, 40:64) hit zero rows of wout so any finite value works.
                nc.vector.memset(HTb[m][:], 1.0)

            # ---- phase 1: embedding gather -> encT -> P20 ----
            with (
                tc.tile_pool(name="gath", bufs=2) as gpool,
                tc.tile_pool(name="pst", bufs=1, space="PSUM") as pstp,
            ):
                for g in range(NT):
                    encg = gpool.tile([128, EMB], dt.float32, tag="encg")
                    nc.gpsimd.indirect_dma_start(
                        out=encg[:],
                        out_offset=None,
                        in_=emb_d.ap(),
                        in_offset=bass.IndirectOffsetOnAxis(ap=idx_sb[:, g : g + 1], axis=0),
                    )
                    pst = pstp.tile([EMB, 128], dt.float32, tag="pst")
                    nc.tensor.transpose(out=pst[:], in_=encg[:], identity=ident_sb[:])
                    nc.vector.tensor_copy(encT[0:EMB, g * 128 : (g + 1) * 128], pst[:])
                p20ps = pstp.tile([42, TOK], dt.float32, tag="p20")
                nc.tensor.matmul(p20ps[:], lhsT=wea_sb[:], rhs=encT[:], start=True, stop=True)
                nc.vector.tensor_copy(P20[:], p20ps[:])

            # ---- phase 2: the two GRU scans, interleaved, 63 steps ----
            # state S [106, BS]: rows 0:8 fwd h, 32:40 bwd h,
            # 64:66 = fwd ez,er; 66:74 = fwd eh; 96:98 = bwd ez,er;
            # 98:106 = bwd eh.  wzr output is quadrant-replicated:
            # cols 0:32 = z1, 32:64 = z2, 64:96 = r1, 96:128 = r2, all
            # pre-scaled by 0.5 so tanh gives y with z = (1+y)/2.
            # whh output: cols 0:64 = G' = 0.5*(Whh.T h) spread, cols
            # 64:128 = G' + eh (selector rows add the e-terms), so
            # cand = tanh(y_r * G' + (G' + eh)) = tanh(r*(Whh.T h) + eh).
            zrpsp = tc.alloc_tile_pool(name="zrps", bufs=1, space="PSUM")
            g2psp = tc.alloc_tile_pool(name="g2ps", bufs=1, space="PSUM")
            prjp = tc.alloc_tile_pool(name="prj", bufs=3, space="PSUM")

            S = spool.tile([106, BS], dt.float32, tag="S")
            nc.vector.memset(S[0:64, :], 0.0)
            nc.vector.tensor_copy(S[64:96, :], P20[0:32, 0:BS])
            nc.vector.tensor_copy(S[96:106, :], P20[32:42, (SEQ - 1) * BS : SEQ * BS])
            nc.vector.memset(HT[0][0:8, 0:BS], 0.0)              # fwd state 0 @ block 0
            nc.vector.memset(HT[NT - 1][32:40, 128 - BS : 128], 0.0)  # bwd state 0 @ block 63

            for s in range(SEQ - 1):
                fcol = s * BS               # fwd step s consumes e_t, t = s
                bcol = (SEQ - 1 - s) * BS   # bwd step s consumes e_t, t = 63 - s
                zrps = zrpsp.tile([128, BS], dt.float32, tag="zr")
                nc.tensor.matmul(zrps[:], lhsT=wzr_sb[:], rhs=S[:], start=True, stop=True)
                g2ps = g2psp.tile([128, BS], dt.float32, tag="g2")
                nc.tensor.matmul(g2ps[:], lhsT=whh_sb[:], rhs=S[:], start=True, stop=True)
                y = spool.tile([128, BS], dt.float32, tag="y")
                nc.scalar.activation(out=y[:], in_=zrps[:], func=AF.Tanh)
                # r-path (critical): cand = tanh(y_r*G' + (G'+eh))
                m2 = spool.tile([64, BS], dt.float32, tag="m2")
                nc.vector.tensor_mul(m2[:], y[64:128, :], g2ps[0:64, :])
                nc.vector.tensor_add(g2ps[64:128, :], g2ps[64:128, :], m2[:])
                cand = spool.tile([64, BS], dt.float32, tag="cand")
                nc.scalar.activation(out=cand[:], in_=g2ps[64:128, :], func=AF.Tanh)
                # z-path (off critical path, runs while ACT computes cand):
                # zh = (1+y_z)/2;  p = h - zh*h
                zh = spool.tile([64, BS], dt.float32, tag="zh")
                nc.vector.tensor_scalar(zh[:], y[0:64, :], 0.5, 0.5, ALU.mult, ALU.add)
                t1 = spool.tile([64, BS], dt.float32, tag="t1")
                nc.vector.tensor_mul(t1[:], S[0:64, :], zh[:])
                p = spool.tile([64, BS], dt.float32, tag="p")
                nc.vector.tensor_sub(p[:], S[0:64, :], t1[:])
                # h_new = p + zh*cand
                w = spool.tile([64, BS], dt.float32, tag="w")
                nc.vector.tensor_mul(w[:], cand[:], zh[:])
                S2 = spool.tile([106, BS], dt.float32, tag="S")
                nc.vector.tensor_add(S2[0:64, :], p[:], w[:])
                # load next step's input-side terms (static data, off the
                # critical path)
                nc.vector.tensor_copy(S2[64:96, :], P20[0:32, fcol + BS : fcol + 2 * BS])
                nc.vector.tensor_copy(S2[96:106, :], P20[32:42, bcol - BS : bcol])
                # store pre-update states: fwd block s+1, bwd block 62-s
                fb = s + 1
                bb = SEQ - 2 - s
                nc.gpsimd.tensor_copy(HT[fb // 16][0:8, (fb % 16) * BS : (fb % 16) * BS + BS],
                                      S2[0:8, :])
                nc.gpsimd.tensor_copy(HT[bb // 16][32:40, (bb % 16) * BS : (bb % 16) * BS + BS],
                                      S2[32:40, :])
                S = S2

            # ---- phase 3: projection + log_softmax, software-pipelined ----
            # proj tiles in readiness order; pass 1 (exp+accum on ACT) of tile
            # order[i] is interleaved with pass 2 (subtract+stage on DVE) of
            # tile order[i-1] so all engines + the output DMA overlap.
            proj_order = (1, 2, 0, 3)

            def prep_tile(m):
                nc.vector.tensor_copy(HTb[m][0:8, :], HT[m][0:8, :])
                nc.vector.tensor_copy(HTb[m][32:40, :], HT[m][32:40, :])
                sums = smp.tile([128, NG], dt.float32, tag=f"sums{m}")
                return sums

            def p1_group(m, g, sums):
                lps = prjp.tile([128, 2, 512], dt.float32, tag="lps")
                for h in range(2):
                    j = 2 * g + h
                    nc.tensor.matmul(lps[:, h, 0:NCHUNK], lhsT=HTb[m][:],
                                     rhs=wout_sb[:, j * NCHUNK : (j + 1) * NCHUNK],
                                     start=True, stop=True)
                esc = escp.tile([128, 2, 512], dt.bfloat16, tag="esc")
                nc.scalar.activation(out=esc[:, :, 0:NCHUNK], in_=lps[:, :, 0:NCHUNK],
                                     func=AF.Exp, accum_out=sums[:, g : g + 1])

            def finish_nlz(m, sums):
                nlz = smp.tile([128, 2], dt.float32, tag=f"nlz{m}")
                nc.vector.reduce_sum(out=nlz[:, 0:1], in_=sums[:, 0:NG],
                                     axis=mybir.AxisListType.X)
                nc.scalar.activation(out=nlz[:, 1:2], in_=nlz[:, 0:1], func=AF.Ln)
                nc.vector.tensor_scalar_mul(nlz[:, 0:1], nlz[:, 1:2], -1.0)
                return nlz

            def p2_group(m, g, stg, nlz, drain_act=False):
                lps = prjp.tile([128, 2, 512], dt.float32, tag="lps")
                for h in range(2):
                    j = 2 * g + h
                    nc.tensor.matmul(lps[:, h, 0:NCHUNK], lhsT=HTb[m][:],
                                     rhs=wout_sb[:, j * NCHUNK : (j + 1) * NCHUNK],
                                     start=True, stop=True)
                gg = g % PIECE_G
                dst = stg[:, 2 * gg * NCHUNK : (2 * gg + 2) * NCHUNK]
                dst = dst.rearrange("p (two c) -> p two c", two=2)
                if drain_act:
                    nc.scalar.add(dst, lps[:, :, 0:NCHUNK], nlz[:, 0:1])
                else:
                    nc.vector.tensor_scalar_add(dst, lps[:, :, 0:NCHUNK], nlz[:, 0:1])

            def p2_piece_dma(m, piece, stg):
                nc.sync.dma_start(
                    out_d[m * 128 : (m + 1) * 128,
                          piece * PIECE_G * 2 * NCHUNK : (piece + 1) * PIECE_G * 2 * NCHUNK],
                    stg[:],
                )

            # phase A: pass 1 of the first tile alone (ACT-bound)
            first = proj_order[0]
            sums_cur = prep_tile(first)
            for g in range(NG):
                p1_group(first, g, sums_cur)
            nlz_prev = finish_nlz(first, sums_cur)

            # middle phases: pass 2 of prev tile + pass 1 of cur tile
            for i in range(1, NT):
                cur, prev = proj_order[i], proj_order[i - 1]
                sums_cur = prep_tile(cur)
                for piece in range(NG // PIECE_G):
                    stg = stgp.tile([128, PIECE_G * 2 * NCHUNK], dt.float32, tag="stg")
                    for gg in range(PIECE_G):
                        g = piece * PIECE_G + gg
                        p2_group(prev, g, stg, nlz_prev)
                        p1_group(cur, g, sums_cur)
                    p2_piece_dma(prev, piece, stg)
                nlz_prev = finish_nlz(cur, sums_cur)

            # final phase: pass 2 of the last tile alone (split ACT/DVE)
            last = proj_order[NT - 1]
            for piece in range(NG // PIECE_G):
                stg = stgp.tile([128, PIECE_G * 2 * NCHUNK], dt.float32, tag="stg")
                for gg in range(PIECE_G):
                    g = piece * PIECE_G + gg
                    p2_group(last, g, stg, nlz_prev, drain_act=(gg % 2 == 1))
                p2_piece_dma(last, piece, stg)

            for pool in (prjp, g2psp, zrpsp):
                pool.release()

    nc.compile()
    return nc


def _prep_weights(embeddings, Wz1, bz1, Wr1, br1, Wh1, bh1, Wz2, bz2, Wr2, br2, Wh2, bh2,
                  Wout, bout):
    f32 = np.float32
    emb = np.ascontiguousarray(np.asarray(embeddings, dtype=f32))
    vocab = emb.shape[0]

    Wz1, Wr1, Wh1 = (np.asarray(a, dtype=f32) for a in (Wz1, Wr1, Wh1))
    Wz2, Wr2, Wh2 = (np.asarray(a, dtype=f32) for a in (Wz2, Wr2, Wh2))

    # We_all [33, 42]: embedding-side weights for all gates, bias row folded
    # in, columns in P20 row layout: 0=z1, 1=r1, 2:10=h1, 32=z2, 33=r2,
    # 34:42=h2.  cat = [h, e].
    wea = np.zeros((EMB + 1, 42), dtype=f32)
    wea[:EMB, 0] = Wz1[HID:, 0]
    wea[:EMB, 1] = Wr1[HID:, 0]
    wea[:EMB, 2:10] = Wh1[HID:, :]
    wea[:EMB, 32] = Wz2[HID:, 0]
    wea[:EMB, 33] = Wr2[HID:, 0]
    wea[:EMB, 34:42] = Wh2[HID:, :]
    wea[EMB, 0] = np.asarray(bz1)[0]
    wea[EMB, 1] = np.asarray(br1)[0]
    wea[EMB, 2:10] = np.asarray(bh1)
    wea[EMB, 32] = np.asarray(bz2)[0]
    wea[EMB, 33] = np.asarray(br2)[0]
    wea[EMB, 34:42] = np.asarray(bh2)

    # Wzr [106, 128]: hidden-side z/r weights plus selector rows that pass
    # through the precomputed input-side terms carried in S rows 64:106.
    # Columns quadrant-replicated (so the PSUM output is already the
    # broadcast gate plane) and pre-scaled by 0.5 for the tanh sigmoid.
    wzr = np.zeros((106, 128), dtype=f32)
    for j in range(32):
        wzr[0:HID, j] = 0.5 * Wz1[:HID, 0]        # z1 plane
        wzr[64, j] = 0.5
        wzr[32 : 32 + HID, 32 + j] = 0.5 * Wz2[:HID, 0]  # z2 plane
        wzr[96, 32 + j] = 0.5
        wzr[0:HID, 64 + j] = 0.5 * Wr1[:HID, 0]   # r1 plane
        wzr[65, 64 + j] = 0.5
        wzr[32 : 32 + HID, 96 + j] = 0.5 * Wr2[:HID, 0]  # r2 plane
        wzr[97, 96 + j] = 0.5

    # Whh [106, 128]: cols 0:64 = G' = 0.5*(Whh.T h) in spread layout; cols
    # 64:128 = G' + eh via selector rows.
    whh = np.zeros((106, 128), dtype=f32)
    whh[0:HID, 0:HID] = 0.5 * Wh1[:HID, :]
    whh[32 : 32 + HID, 32 : 32 + HID] = 0.5 * Wh2[:HID, :]
    whh[0:HID, 64 : 64 + HID] = 0.5 * Wh1[:HID, :]
    whh[32 : 32 + HID, 96 : 96 + HID] = 0.5 * Wh2[:HID, :]
    for k in range(HID):
        whh[66 + k, 64 + k] = 1.0
        whh[98 + k, 96 + k] = 1.0

    # Wout_aug [65, vocab] bf16: rows 0:8 fwd-h weights, 32:40 bwd-h
    # weights, 64 = bout; all other rows zero (matching HTb junk lanes).
    Wout = np.asarray(Wout, dtype=f32)
    wout_aug = np.zeros((65, vocab), dtype=f32)
    wout_aug[0:HID, :] = Wout[0:HID, :]
    wout_aug[32 : 32 + HID, :] = Wout[HID:, :]
    wout_aug[64, :] = np.asarray(bout, dtype=f32)
    wout_aug = wout_aug.astype(ml_dtypes.bfloat16)

    return dict(emb=emb, wea=wea, wzr=wzr, whh=whh, wout=wout_aug,
                vocab=vocab)


def run(inputs, trace=False):
    from concourse.bass_utils import run_bass_kernel_spmd

    w = _prep_weights(
        inputs["embeddings"],
        inputs["Wz1"], inputs["bz1"], inputs["Wr1"], inputs["br1"],
        inputs["Wh1"], inputs["bh1"],
        inputs["Wz2"], inputs["bz2"], inputs["Wr2"], inputs["br2"],
        inputs["Wh2"], inputs["bh2"],
        inputs["Wout"], inputs["bout"],
    )
    vocab = w.pop("vocab")
    x = np.ascontiguousarray(np.asarray(inputs["x"], dtype=np.int32))
    assert x.shape == (SEQ, BATCH)

    key = ("module", vocab)
    if key not in _module_cache:
        _module_cache[key] = _build_module(vocab=vocab)
    nc = _module_cache[key]

    in_maps = []
    for c in range(NCORES):
        m = dict(w)
        m["x"] = np.ascontiguousarray(x[:, c * BS : (c + 1) * BS])
        in_maps.append(m)

    res = run_bass_kernel_spmd(nc, in_maps, core_ids=list(range(NCORES)), trace=trace)
    shards = [res.results[c]["out"].reshape(SEQ, BS, vocab) for c in range(NCORES)]
    out = np.concatenate(shards, axis=1)
    return out, res


def kernel(**inputs):
    out, _ = run(inputs)
    return out


# revision 5
# speedup vs baseline: 1.3129x; 1.2149x over previous
"""Trainium2 Bass kernel for a bidirectional GRU language model head.

Model (see problem reference): tokens x[T=64, B=64] -> embedding[32000, 32]
-> forward GRU (H=8, scalar z/r gates) + backward GRU -> concat [T,B,16]
-> logits = h @ Wout[16, 32000] + bout -> log_softmax over vocab.

Output is [64, 64, 32000] f32 = 512 MB. Sharding: data-parallel over batch;
core c gets batch columns [8c, 8c+8), runs the full T=64 recurrence for its
8 sequences and the full-vocab projection + log-softmax for its 512 tokens.
No collectives.

Key performance structure (per core):
  - The 63-step recurrence is a serial dependency chain; everything is done
    to keep its per-step latency minimal: both directions run merged in one
    quadrant-spread tile set, all scan weights and state are BF16 (an fp32
    stationary with >64 contraction rows splits every matmul into two
    LDWEIGHTS+MATMUL pairs on the PE - the dominant cost of a previous
    version), gates use TANH only (z = (1+tanh(a/2))/2 with the 1/2 folded
    into the weights) so the ACT table set never switches, the gate matmul
    emits already-broadcast gate planes (weight columns replicated 8-wide),
    and the input-side candidate terms ride selector rows of the candidate
    matmul.
  - Projection per 128-token tile: logits = HTb.T @ Wout_aug (K=65 bf16,
    bias via ones lane), in 2x500-column chunk groups (2-bank PSUM
    group per pair). Pass 1: ACT
    exp+accumulate drains PSUM and yields sum(exp) (no max-shift needed,
    |logits| <= 4.25). log(sum) is computed on the DVE (exponent-field
    extraction + polynomial) so the ACT function table never leaves the
    exp/tanh set. Pass 2 recomputes the matmul and subtracts logsumexp
    while draining to a staging buffer (DVE, every 4th chunk on ACT),
    DMA'd out in 4 MB pieces. Pass 1 of tile m+1 interleaves with pass 2
    of tile m so ACT (exp), DVE (drain), PE (matmuls) and the output DMA
    all run concurrently; the Tile scheduler also overlaps the first
    tile's pass 1 with the scan tail.
"""

import numpy as np
import ml_dtypes

VOCAB, HID, EMB = 32000, 8, 32
SEQ, BATCH = 64, 64
NCORES = 8
BS = BATCH // NCORES          # batch columns per core
TOK = SEQ * BS                # tokens per core
NCHUNK = 500                  # vocab columns per matmul (PSUM bank = 512 f32)

_module_cache = {}

# ln(1+t) on [0,1], degree-5 least-squares fit (max err ~2e-5)
_LN_C = [3.01026250e-02, -1.30119415e-01, 2.83304325e-01, -4.89156847e-01,
         9.99010447e-01, 2.21170312e-05]
_LN2 = float(np.log(2.0))


def _build_module(vocab=VOCAB):
    import concourse.bass as bass
    import concourse.bacc as bacc
    import concourse.mybir as mybir
    import concourse.tile as tile
    from concourse.masks import make_identity

    dt = mybir.dt
    AF = mybir.ActivationFunctionType
    ALU = mybir.AluOpType

    NG = vocab // (2 * NCHUNK)     # 32 chunk-groups per 128-token tile
    assert NG * 2 * NCHUNK == vocab
    PIECE_G = 8                    # groups per staged DMA piece (8000 cols, 4 MB)
    assert NG % PIECE_G == 0

    nc = bacc.Bacc("TRN2", target_bir_lowering=False, debug=False)

    x_d = nc.dram_tensor("x", [SEQ, BS], dt.int32, kind="ExternalInput")
    emb_d = nc.dram_tensor("emb", [vocab, EMB], dt.float32, kind="ExternalInput")
    wea_d = nc.dram_tensor("wea", [EMB + 1, 42], dt.float32, kind="ExternalInput")
    wzr_d = nc.dram_tensor("wzr", [106, 104], dt.bfloat16, kind="ExternalInput")
    whh_d = nc.dram_tensor("whh", [106, 128], dt.bfloat16, kind="ExternalInput")
    wout_d = nc.dram_tensor("wout", [65, vocab], dt.bfloat16, kind="ExternalInput")
    out_d = nc.dram_tensor("out", [TOK, vocab], dt.float32, kind="ExternalOutput")

    NT = TOK // 128  # 128-token projection tiles (4)

    with tile.TileContext(nc) as tc:
        with (
            tc.tile_pool(name="const", bufs=1) as cpool,
            tc.tile_pool(name="scan", bufs=2) as spool,
            tc.tile_pool(name="stage", bufs=3) as stgp,
            tc.tile_pool(name="small", bufs=1) as smp,
            tc.tile_pool(name="expsc", bufs=2) as escp,
        ):
            # ---- constants / inputs to SBUF (small ones first: the scan is
            # gated on them; wout is only needed ~100us later) ----
            idx_sb = cpool.tile([128, NT], dt.int32)
            # token g*128+p lives at x[(g*16 + p//8), p%8]
            nc.sync.dma_start(idx_sb[:], x_d.ap().rearrange("(g q) b -> (q b) g", g=NT))
            wea_sb = cpool.tile([EMB + 1, 42], dt.float32)
            nc.sync.dma_start(wea_sb[:], wea_d[:])
            wzr_sb = cpool.tile([106, 104], dt.bfloat16)
            nc.sync.dma_start(wzr_sb[:], wzr_d[:])
            whh_sb = cpool.tile([106, 128], dt.bfloat16)
            nc.sync.dma_start(whh_sb[:], whh_d[:])
            wout_sb = cpool.tile([65, vocab], dt.bfloat16)
            nc.sync.dma_start(wout_sb[:], wout_d[:])
            ident_sb = cpool.tile([128, 128], dt.float32)
            make_identity(nc, ident_sb[:])

            encT = cpool.tile([EMB + 1, TOK], dt.float32)
            nc.vector.memset(encT[EMB : EMB + 1, :], 1.0)
            # P20 rows: 0=ez1, 1=er1, 2:10=eh1 (fwd, token order); 10:32 zero;
            # 32=ez2, 33=er2, 34:42=eh2 (bwd).  Biases folded via encT ones row.
            P20 = cpool.tile([42, TOK], dt.float32)
            HT = [cpool.tile([40, 128], dt.bfloat16, name=f"HT{m}", tag=f"HT{m}")
                  for m in range(NT)]
            HTb = [cpool.tile([65, 128], dt.bfloat16, name=f"HTb{m}", tag=f"HTb{m}")
                   for m in range(NT)]
            for m in range(NT):
                # 1.0 everywhere: row 64 is the bias ones-row; unused lanes
                # (8:32, 40:64) hit zero rows of wout so any finite value works.
                nc.vector.memset(HTb[m][:], 1.0)

            # ---- phase 1: embedding gather -> encT -> P20 ----
            with (
                tc.tile_pool(name="gath", bufs=2) as gpool,
                tc.tile_pool(name="pst", bufs=1, space="PSUM") as pstp,
            ):
                for g in range(NT):
                    encg = gpool.tile([128, EMB], dt.float32, tag="encg")
                    nc.gpsimd.indirect_dma_start(
                        out=encg[:],
                        out_offset=None,
                        in_=emb_d.ap(),
                        in_offset=bass.IndirectOffsetOnAxis(ap=idx_sb[:, g : g + 1], axis=0),
                    )
                    pst = pstp.tile([EMB, 128], dt.float32, tag="pst")
                    nc.tensor.transpose(out=pst[:], in_=encg[:], identity=ident_sb[:])
                    nc.vector.tensor_copy(encT[0:EMB, g * 128 : (g + 1) * 128], pst[:])
                p20ps = pstp.tile([42, TOK], dt.float32, tag="p20")
                nc.tensor.matmul(p20ps[:], lhsT=wea_sb[:], rhs=encT[:], start=True, stop=True)
                nc.vector.tensor_copy(P20[:], p20ps[:])

            # ---- phase 2: the two GRU scans, interleaved, 63 steps ----
            # state S [106, BS] bf16: rows 0:8 fwd h, 32:40 bwd h, 40:64 zero,
            # 64:66 = fwd ez,er; 66:74 = fwd eh; 96:98 = bwd ez,er;
            # 98:106 = bwd eh.  wzr out cols (x0.5, tanh sigmoid): 0:8 = z1
            # (8-wide so the plane is pre-broadcast), 32:40 = z2, 64:72 = r1,
            # 96:104 = r2.  whh out cols: 0:8/32:40 = G' = 0.5*(Whh.T h);
            # 64:72/96:104 = G' + eh (selector rows), so
            # cand = tanh(y_r*G' + (G'+eh)) = tanh(r*(Whh.T h) + eh).
            zrpsp = tc.alloc_tile_pool(name="zrps", bufs=1, space="PSUM")
            g2psp = tc.alloc_tile_pool(name="g2ps", bufs=1, space="PSUM")
            prjp = tc.alloc_tile_pool(name="prj", bufs=3, space="PSUM")

            S = spool.tile([106, BS], dt.bfloat16, tag="S")
            nc.vector.memset(S[0:64, :], 0.0)
            nc.vector.tensor_copy(S[64:96, :], P20[0:32, 0:BS])
            nc.vector.tensor_copy(S[96:106, :], P20[32:42, (SEQ - 1) * BS : SEQ * BS])
            nc.vector.memset(HT[0][0:8, 0:BS], 0.0)              # fwd state 0 @ block 0
            nc.vector.memset(HT[NT - 1][32:40, 128 - BS : 128], 0.0)  # bwd state 0 @ block 63

            for s in range(SEQ - 1):
                fcol = s * BS               # fwd step s consumes e_t, t = s
                bcol = (SEQ - 1 - s) * BS   # bwd step s consumes e_t, t = 63 - s
                zrps = zrpsp.tile([104, BS], dt.float32, tag="zr")
                nc.tensor.matmul(zrps[:], lhsT=wzr_sb[:], rhs=S[:], start=True, stop=True)
                g2ps = g2psp.tile([128, BS], dt.float32, tag="g2")
                nc.tensor.matmul(g2ps[:], lhsT=whh_sb[:], rhs=S[:], start=True, stop=True)
                S2 = spool.tile([106, BS], dt.bfloat16, tag="S")
                # next step's input-side terms: static data, off the critical
                # path, emitted first so they never gate the chain
                nc.vector.tensor_copy(S2[64:96, :], P20[0:32, fcol + BS : fcol + 2 * BS])
                nc.vector.tensor_copy(S2[96:106, :], P20[32:42, bcol - BS : bcol])
                y = spool.tile([104, BS], dt.float32, tag="y")
                nc.scalar.activation(out=y[:], in_=zrps[:], func=AF.Tanh)
                # r-path (critical): cand = tanh(y_r*G' + (G'+eh))
                m2 = spool.tile([40, BS], dt.float32, tag="m2")
                nc.vector.tensor_mul(m2[:], y[64:104, :], g2ps[0:40, :])
                nc.vector.tensor_add(g2ps[64:104, :], g2ps[64:104, :], m2[:])
                cand = spool.tile([64, BS], dt.float32, tag="cand")
                nc.scalar.activation(out=cand[:], in_=g2ps[64:128, :], func=AF.Tanh)
                # z-path (off critical path, runs while ACT computes cand):
                # zh = (1+y_z)/2;  p = h - zh*h
                zh = spool.tile([64, BS], dt.float32, tag="zh")
                nc.vector.tensor_scalar(zh[:], y[0:64, :], 0.5, 0.5, ALU.mult, ALU.add)
                t1 = spool.tile([64, BS], dt.float32, tag="t1")
                nc.vector.tensor_mul(t1[:], S[0:64, :], zh[:])
                p = spool.tile([64, BS], dt.float32, tag="p")
                nc.vector.tensor_sub(p[:], S[0:64, :], t1[:])
                # h_new = p + zh*cand
                w = spool.tile([64, BS], dt.float32, tag="w")
                nc.vector.tensor_mul(w[:], cand[:], zh[:])
                nc.vector.tensor_add(S2[0:64, :], p[:], w[:])
                # store pre-update states: fwd block s+1, bwd block 62-s
                fb = s + 1
                bb = SEQ - 2 - s
                nc.gpsimd.tensor_copy(HT[fb // 16][0:8, (fb % 16) * BS : (fb % 16) * BS + BS],
                                      S2[0:8, :])
                nc.gpsimd.tensor_copy(HT[bb // 16][32:40, (bb % 16) * BS : (bb % 16) * BS + BS],
                                      S2[32:40, :])
                S = S2

            # ---- phase 3: projection + log_softmax, software-pipelined ----
            proj_order = (1, 2, 0, 3)

            def prep_tile(m):
                nc.vector.tensor_copy(HTb[m][0:8, :], HT[m][0:8, :])
                nc.vector.tensor_copy(HTb[m][32:40, :], HT[m][32:40, :])
                sums = smp.tile([128, NG], dt.float32, tag=f"sums{m}")
                return sums

            def p1_group(m, g, sums):
                lps = prjp.tile([128, 2, 512], dt.float32, tag="lps")
                for h in range(2):
                    j = 2 * g + h
                    nc.tensor.matmul(lps[:, h, 0:NCHUNK], lhsT=HTb[m][:],
                                     rhs=wout_sb[:, j * NCHUNK : (j + 1) * NCHUNK],
                                     start=True, stop=True)
                esc = escp.tile([128, 2, 512], dt.bfloat16, tag="esc")
                nc.scalar.activation(out=esc[:, :, 0:NCHUNK], in_=lps[:, :, 0:NCHUNK],
                                     func=AF.Exp, accum_out=sums[:, g : g + 1])

            def finish_nlz(m, sums):
                # nlz = -ln(sum(exp)) computed on DVE (exponent extraction +
                # ln(1+t) polynomial) so the ACT table set never switches.
                ws = smp.tile([128, 8], dt.float32, tag=f"nlzw{m}")
                wi = smp.tile([128, 2], dt.int32, tag=f"nlzi{m}")
                nc.vector.reduce_sum(out=ws[:, 0:1], in_=sums[:, 0:NG],
                                     axis=mybir.AxisListType.X)
                bits = ws[:, 0:1].bitcast(dt.int32)
                # exponent as float without an int->float cast: (bits>>23)
                # OR 0x4B000000 bitcasts to the float 8388608 + e_raw
                nc.vector.tensor_scalar(wi[:, 0:1], bits, 23, 0x4B000000,
                                        ALU.logical_shift_right, ALU.bitwise_or)
                nc.vector.tensor_scalar_sub(ws[:, 1:2], wi[:, 0:1].bitcast(dt.float32),
                                            8388608.0 + 127.0)  # e = e_raw - 127
                nc.vector.tensor_scalar(wi[:, 1:2], bits, 0x007FFFFF, 0x3F800000,
                                        ALU.bitwise_and, ALU.bitwise_or)
                mf = wi[:, 1:2].bitcast(dt.float32)
                nc.vector.tensor_scalar_sub(ws[:, 2:3], mf, 1.0)        # t = m-1
                t_ = ws[:, 2:3]
                pp = ws[:, 3:4]
                nc.vector.tensor_scalar(pp, t_, _LN_C[0], _LN_C[1], ALU.mult, ALU.add)
                for ck in _LN_C[2:]:
                    nc.vector.tensor_mul(pp, pp, t_)
                    nc.vector.tensor_scalar_add(pp, pp, float(ck))
                # lse = e*ln2 + ln(m);  nlz = -lse
                nc.vector.scalar_tensor_tensor(ws[:, 4:5], ws[:, 1:2], _LN2, pp,
                                               ALU.mult, ALU.add)
                nc.vector.tensor_scalar_mul(ws[:, 5:6], ws[:, 4:5], -1.0)
                return ws[:, 5:6]

            def p2_group(m, g, stg, nlz, drain_act=False):
                lps = prjp.tile([128, 2, 512], dt.float32, tag="lps")
                for h in range(2):
                    j = 2 * g + h
                    nc.tensor.matmul(lps[:, h, 0:NCHUNK], lhsT=HTb[m][:],
                                     rhs=wout_sb[:, j * NCHUNK : (j + 1) * NCHUNK],
                                     start=True, stop=True)
                gg = g % PIECE_G
                dst = stg[:, 2 * gg * NCHUNK : (2 * gg + 2) * NCHUNK]
                dst = dst.rearrange("p (two c) -> p two c", two=2)
                if drain_act:
                    nc.scalar.add(dst, lps[:, :, 0:NCHUNK], nlz)
                else:
                    nc.vector.tensor_scalar_add(dst, lps[:, :, 0:NCHUNK], nlz)

            def p2_piece_dma(m, piece, stg):
                nc.sync.dma_start(
                    out_d[m * 128 : (m + 1) * 128,
                          piece * PIECE_G * 2 * NCHUNK : (piece + 1) * PIECE_G * 2 * NCHUNK],
                    stg[:],
                )

            # phase A: pass 1 of the first tile alone (overlaps the scan tail)
            first = proj_order[0]
            sums_cur = prep_tile(first)
            for g in range(NG):
                p1_group(first, g, sums_cur)
            nlz_prev = finish_nlz(first, sums_cur)

            # middle phases: pass 2 of prev tile + pass 1 of cur tile
            for i in range(1, NT):
                cur, prev = proj_order[i], proj_order[i - 1]
                sums_cur = prep_tile(cur)
                for piece in range(NG // PIECE_G):
                    stg = stgp.tile([128, PIECE_G * 2 * NCHUNK], dt.float32, tag="stg")
                    for gg in range(PIECE_G):
                        g = piece * PIECE_G + gg
                        p2_group(prev, g, stg, nlz_prev, drain_act=(gg % 4 == 3))
                        p1_group(cur, g, sums_cur)
                    p2_piece_dma(prev, piece, stg)
                nlz_prev = finish_nlz(cur, sums_cur)

            # final phase: pass 2 of the last tile alone (split ACT/DVE)
            last = proj_order[NT - 1]
            for piece in range(NG // PIECE_G):
                stg = stgp.tile([128, PIECE_G * 2 * NCHUNK], dt.float32, tag="stg")
                for gg in range(PIECE_G):
                    g = piece * PIECE_G + gg
                    p2_group(last, g, stg, nlz_prev, drain_act=(gg % 2 == 1))
                p2_piece_dma(last, piece, stg)

            for pool in (prjp, g2psp, zrpsp):
                pool.release()

    nc.compile()
    return nc


def _prep_weights(embeddings, Wz1, bz1, Wr1, br1, Wh1, bh1, Wz2, bz2, Wr2, br2, Wh2, bh2,
                  Wout, bout):
    f32 = np.float32
    bf16 = ml_dtypes.bfloat16
    emb = np.ascontiguousarray(np.asarray(embeddings, dtype=f32))
    vocab = emb.shape[0]

    Wz1, Wr1, Wh1 = (np.asarray(a, dtype=f32) for a in (Wz1, Wr1, Wh1))
    Wz2, Wr2, Wh2 = (np.asarray(a, dtype=f32) for a in (Wz2, Wr2, Wh2))

    # We_all [33, 42]: embedding-side weights for all gates, bias row folded
    # in, columns in P20 row layout: 0=z1, 1=r1, 2:10=h1, 32=z2, 33=r2,
    # 34:42=h2.  cat = [h, e].
    wea = np.zeros((EMB + 1, 42), dtype=f32)
    wea[:EMB, 0] = Wz1[HID:, 0]
    wea[:EMB, 1] = Wr1[HID:, 0]
    wea[:EMB, 2:10] = Wh1[HID:, :]
    wea[:EMB, 32] = Wz2[HID:, 0]
    wea[:EMB, 33] = Wr2[HID:, 0]
    wea[:EMB, 34:42] = Wh2[HID:, :]
    wea[EMB, 0] = np.asarray(bz1)[0]
    wea[EMB, 1] = np.asarray(br1)[0]
    wea[EMB, 2:10] = np.asarray(bh1)
    wea[EMB, 32] = np.asarray(bz2)[0]
    wea[EMB, 33] = np.asarray(br2)[0]
    wea[EMB, 34:42] = np.asarray(bh2)

    # Wzr [106, 104] bf16: hidden-side z/r weights (x0.5 for the tanh
    # sigmoid) plus selector rows passing the precomputed input-side terms
    # carried in S rows 64:106.  Out cols replicated 8-wide per gate so the
    # PSUM output is the broadcast gate plane.
    wzr = np.zeros((106, 104), dtype=f32)
    for j in range(HID):
        wzr[0:HID, j] = 0.5 * Wz1[:HID, 0]
        wzr[64, j] = 0.5
        wzr[32 : 32 + HID, 32 + j] = 0.5 * Wz2[:HID, 0]
        wzr[96, 32 + j] = 0.5
        wzr[0:HID, 64 + j] = 0.5 * Wr1[:HID, 0]
        wzr[65, 64 + j] = 0.5
        wzr[32 : 32 + HID, 96 + j] = 0.5 * Wr2[:HID, 0]
        wzr[97, 96 + j] = 0.5

    # Whh [106, 128] bf16: cols 0:8/32:40 = G' = 0.5*(Whh.T h) spread; cols
    # 64:72/96:104 = G' + eh via selector rows; other cols zero.
    whh = np.zeros((106, 128), dtype=f32)
    whh[0:HID, 0:HID] = 0.5 * Wh1[:HID, :]
    whh[32 : 32 + HID, 32 : 32 + HID] = 0.5 * Wh2[:HID, :]
    whh[0:HID, 64 : 64 + HID] = 0.5 * Wh1[:HID, :]
    whh[32 : 32 + HID, 96 : 96 + HID] = 0.5 * Wh2[:HID, :]
    for k in range(HID):
        whh[66 + k, 64 + k] = 1.0
        whh[98 + k, 96 + k] = 1.0

    # Wout_aug [65, vocab] bf16: rows 0:8 fwd-h weights, 32:40 bwd-h
    # weights, 64 = bout; all other rows zero (matching HTb junk lanes).
    Wout = np.asarray(Wout, dtype=f32)
    wout_aug = np.zeros((65, vocab), dtype=f32)
    wout_aug[0:HID, :] = Wout[0:HID, :]
    wout_aug[32 : 32 + HID, :] = Wout[HID:, :]
    wout_aug[64, :] = np.asarray(bout, dtype=f32)

    return dict(emb=emb, wea=wea,
                wzr=wzr.astype(bf16), whh=whh.astype(bf16),
                wout=wout_aug.astype(bf16), vocab=vocab)


def run(inputs, trace=False):
    from concourse.bass_utils import run_bass_kernel_spmd

    w = _prep_weights(
        inputs["embeddings"],
        inputs["Wz1"], inputs["bz1"], inputs["Wr1"], inputs["br1"],
        inputs["Wh1"], inputs["bh1"],
        inputs["Wz2"], inputs["bz2"], inputs["Wr2"], inputs["br2"],
        inputs["Wh2"], inputs["bh2"],
        inputs["Wout"], inputs["bout"],
    )
    vocab = w.pop("vocab")
    x = np.ascontiguousarray(np.asarray(inputs["x"], dtype=np.int32))
    assert x.shape == (SEQ, BATCH)

    key = ("module", vocab)
    if key not in _module_cache:
        _module_cache[key] = _build_module(vocab=vocab)
    nc = _module_cache[key]

    in_maps = []
    for c in range(NCORES):
        m = dict(w)
        m["x"] = np.ascontiguousarray(x[:, c * BS : (c + 1) * BS])
        in_maps.append(m)

    res = run_bass_kernel_spmd(nc, in_maps, core_ids=list(range(NCORES)), trace=trace)
    shards = [res.results[c]["out"].reshape(SEQ, BS, vocab) for c in range(NCORES)]
    out = np.concatenate(shards, axis=1)
    return out, res


def kernel(**inputs):
    out, _ = run(inputs)
    return out


# revision 6
# speedup vs baseline: 1.4714x; 1.1207x over previous
"""Trainium2 Bass kernel for a bidirectional GRU language model head.

Model (see problem reference): tokens x[T=64, B=64] -> embedding[32000, 32]
-> forward GRU (H=8, scalar z/r gates) + backward GRU -> concat [T,B,16]
-> logits = h @ Wout[16, 32000] + bout -> log_softmax over vocab.

Output is [64, 64, 32000] f32 = 512 MB. Sharding: data-parallel over batch;
core c gets batch columns [8c, 8c+8), runs the full T=64 recurrence for its
8 sequences and the full-vocab projection + log-softmax for its 512 tokens.
No collectives.

Key performance structure (per core):
  - The 63-step recurrence is a serial dependency chain; everything is done
    to keep its per-step latency minimal: both directions run merged in one
    quadrant-spread tile set, all scan weights and state are BF16 (an fp32
    stationary with >64 contraction rows splits every matmul into two
    LDWEIGHTS+MATMUL pairs on the PE - the dominant cost of a previous
    version), gates use TANH only (z = (1+tanh(a/2))/2 with the 1/2 folded
    into the weights) so the ACT table set never switches, the gate matmul
    emits already-broadcast gate planes (weight columns replicated 8-wide),
    and the input-side candidate terms ride selector rows of the candidate
    matmul.
  - Projection per 128-token tile: logits = HTb.T @ Wout_aug (K=65 bf16,
    bias via ones lane), in 2x500-column chunk groups (2-bank PSUM
    group per pair). Pass 1: ACT
    exp+accumulate drains PSUM and yields sum(exp) (no max-shift needed,
    |logits| <= 4.25). log(sum) is computed on the DVE (exponent-field
    extraction + polynomial) so the ACT function table never leaves the
    exp/tanh set. Pass 2 recomputes the matmul and subtracts logsumexp
    while draining to a staging buffer (DVE, every 4th chunk on ACT),
    DMA'd out in 4 MB pieces. Pass 1 of tile m+1 interleaves with pass 2
    of tile m so ACT (exp), DVE (drain), PE (matmuls) and the output DMA
    all run concurrently; the Tile scheduler also overlaps the first
    tile's pass 1 with the scan tail.
"""

import numpy as np
import ml_dtypes

VOCAB, HID, EMB = 32000, 8, 32
SEQ, BATCH = 64, 64
NCORES = 8
BS = BATCH // NCORES          # batch columns per core
TOK = SEQ * BS                # tokens per core
NCHUNK = 500                  # vocab columns per matmul (PSUM bank = 512 f32)

_module_cache = {}

# ln(1+t) on [0,1], degree-5 least-squares fit (max err ~2e-5)
_LN_C = [3.01026250e-02, -1.30119415e-01, 2.83304325e-01, -4.89156847e-01,
         9.99010447e-01, 2.21170312e-05]
_LN2 = float(np.log(2.0))


def _build_module(vocab=VOCAB):
    import concourse.bass as bass
    import concourse.bacc as bacc
    import concourse.mybir as mybir
    import concourse.tile as tile
    from concourse.masks import make_identity

    dt = mybir.dt
    AF = mybir.ActivationFunctionType
    ALU = mybir.AluOpType

    NG = vocab // (2 * NCHUNK)     # 32 chunk-groups per 128-token tile
    assert NG * 2 * NCHUNK == vocab
    PIECE_G = 8                    # groups per staged DMA piece (8000 cols, 4 MB)
    assert NG % PIECE_G == 0

    nc = bacc.Bacc("TRN2", target_bir_lowering=False, debug=False)

    x_d = nc.dram_tensor("x", [SEQ, BS], dt.int32, kind="ExternalInput")
    emb_d = nc.dram_tensor("emb", [vocab, EMB], dt.float32, kind="ExternalInput")
    wea_d = nc.dram_tensor("wea", [EMB + 1, 42], dt.float32, kind="ExternalInput")
    wzr_d = nc.dram_tensor("wzr", [106, 104], dt.bfloat16, kind="ExternalInput")
    whh_d = nc.dram_tensor("whh", [106, 128], dt.bfloat16, kind="ExternalInput")
    wout_d = nc.dram_tensor("wout", [128, vocab], dt.bfloat16, kind="ExternalInput")
    out_d = nc.dram_tensor("out", [TOK, vocab], dt.float32, kind="ExternalOutput")

    NT = TOK // 128  # 128-token projection tiles (4)

    with tile.TileContext(nc) as tc:
        with (
            tc.tile_pool(name="const", bufs=1) as cpool,
            tc.tile_pool(name="scan", bufs=2) as spool,
            tc.tile_pool(name="stage", bufs=3) as stgp,
            tc.tile_pool(name="small", bufs=1) as smp,
            tc.tile_pool(name="expsc", bufs=2) as escp,
        ):
            # ---- constants / inputs to SBUF (small ones first: the scan is
            # gated on them; wout is only needed ~100us later) ----
            idx_sb = cpool.tile([128, NT], dt.int32)
            # token g*128+p lives at x[(g*16 + p//8), p%8]
            nc.sync.dma_start(idx_sb[:], x_d.ap().rearrange("(g q) b -> (q b) g", g=NT))
            wea_sb = cpool.tile([EMB + 1, 42], dt.float32)
            nc.sync.dma_start(wea_sb[:], wea_d[:])
            wzr_sb = cpool.tile([106, 104], dt.bfloat16)
            nc.sync.dma_start(wzr_sb[:], wzr_d[:])
            whh_sb = cpool.tile([106, 128], dt.bfloat16)
            nc.sync.dma_start(whh_sb[:], whh_d[:])
            wout_sb = cpool.tile([128, vocab], dt.bfloat16)
            nc.sync.dma_start(wout_sb[:], wout_d[:])
            ident_sb = cpool.tile([128, 128], dt.float32)
            make_identity(nc, ident_sb[:])

            encT = cpool.tile([EMB + 1, TOK], dt.float32)
            nc.vector.memset(encT[EMB : EMB + 1, :], 1.0)
            # P20Q holds the per-token input-side gate terms, pre-placed at
            # the quadrant rows S wants (64:66=ez1,er1; 66:74=eh1; 74:96
            # zero; 96:98=ez2,er2; 98:106=eh2) and in bf16, so the per-step
            # state loads are partition-local gpsimd copies off the DVE.
            P20Q = cpool.tile([106, TOK], dt.bfloat16)
            HT = [cpool.tile([40, 128], dt.bfloat16, name=f"HT{m}", tag=f"HT{m}")
                  for m in range(NT)]
            HTb = [cpool.tile([128, 128], dt.bfloat16, name=f"HTb{m}", tag=f"HTb{m}")
                   for m in range(NT)]
            for m in range(NT):
                # 1.0 everywhere: row 64 is the bias ones-row; unused lanes
                # (8:32, 40:64) hit zero rows of wout so any finite value works.
                nc.vector.memset(HTb[m][:], 1.0)

            # ---- phase 1: embedding gather -> encT -> P20 ----
            with (
                tc.tile_pool(name="gath", bufs=2) as gpool,
                tc.tile_pool(name="pst", bufs=1, space="PSUM") as pstp,
            ):
                for g in range(NT):
                    encg = gpool.tile([128, EMB], dt.float32, tag="encg")
                    nc.gpsimd.indirect_dma_start(
                        out=encg[:],
                        out_offset=None,
                        in_=emb_d.ap(),
                        in_offset=bass.IndirectOffsetOnAxis(ap=idx_sb[:, g : g + 1], axis=0),
                    )
                    pst = pstp.tile([EMB, 128], dt.float32, tag="pst")
                    nc.tensor.transpose(out=pst[:], in_=encg[:], identity=ident_sb[:])
                    nc.vector.tensor_copy(encT[0:EMB, g * 128 : (g + 1) * 128], pst[:])
                p20ps = pstp.tile([42, TOK], dt.float32, tag="p20")
                nc.tensor.matmul(p20ps[:], lhsT=wea_sb[:], rhs=encT[:], start=True, stop=True)
                nc.vector.tensor_copy(P20Q[64:96, :], p20ps[0:32, :])
                nc.vector.tensor_copy(P20Q[96:106, :], p20ps[32:42, :])

            # ---- phase 2: the two GRU scans, interleaved, 63 steps ----
            # state S [106, BS] bf16: rows 0:8 fwd h, 32:40 bwd h, 40:64 zero,
            # 64:66 = fwd ez,er; 66:74 = fwd eh; 96:98 = bwd ez,er;
            # 98:106 = bwd eh.  wzr out cols (x0.5, tanh sigmoid): 0:8 = z1
            # (8-wide so the plane is pre-broadcast), 32:40 = z2, 64:72 = r1,
            # 96:104 = r2.  whh out cols: 0:8/32:40 = G' = 0.5*(Whh.T h);
            # 64:72/96:104 = G' + eh (selector rows), so
            # cand = tanh(y_r*G' + (G'+eh)) = tanh(r*(Whh.T h) + eh).
            zrpsp = tc.alloc_tile_pool(name="zrps", bufs=1, space="PSUM")
            g2psp = tc.alloc_tile_pool(name="g2ps", bufs=1, space="PSUM")
            prjp = tc.alloc_tile_pool(name="prj", bufs=3, space="PSUM")

            S = spool.tile([106, BS], dt.bfloat16, tag="S")
            nc.vector.memset(S[0:64, :], 0.0)
            nc.vector.tensor_copy(S[64:96, :], P20Q[64:96, 0:BS])
            nc.vector.tensor_copy(S[96:106, :], P20Q[96:106, (SEQ - 1) * BS : SEQ * BS])
            nc.vector.memset(HT[0][0:8, 0:BS], 0.0)              # fwd state 0 @ block 0
            nc.vector.memset(HT[NT - 1][32:40, 128 - BS : 128], 0.0)  # bwd state 0 @ block 63

            for s in range(SEQ - 1):
                fcol = s * BS               # fwd step s consumes e_t, t = s
                bcol = (SEQ - 1 - s) * BS   # bwd step s consumes e_t, t = 63 - s
                zrps = zrpsp.tile([104, BS], dt.float32, tag="zr")
                nc.tensor.matmul(zrps[:], lhsT=wzr_sb[:], rhs=S[:], start=True, stop=True)
                g2ps = g2psp.tile([128, BS], dt.float32, tag="g2")
                nc.tensor.matmul(g2ps[:], lhsT=whh_sb[:], rhs=S[:], start=True, stop=True)
                S2 = spool.tile([106, BS], dt.bfloat16, tag="S")
                # next step's input-side terms: static data, off the critical
                # path, partition-local gpsimd copies (keeps the DVE queue to
                # the 7 ops the chain actually needs)
                nc.gpsimd.tensor_copy(S2[64:96, :], P20Q[64:96, fcol + BS : fcol + 2 * BS])
                nc.gpsimd.tensor_copy(S2[96:106, :], P20Q[96:106, bcol - BS : bcol])
                y = spool.tile([104, BS], dt.float32, tag="y")
                nc.scalar.activation(out=y[:], in_=zrps[:], func=AF.Tanh)
                # r-path (critical): cand = tanh(y_r*G' + (G'+eh))
                m2 = spool.tile([40, BS], dt.float32, tag="m2")
                nc.vector.tensor_mul(m2[:], y[64:104, :], g2ps[0:40, :])
                nc.vector.tensor_add(g2ps[64:104, :], g2ps[64:104, :], m2[:])
                cand = spool.tile([64, BS], dt.float32, tag="cand")
                nc.scalar.activation(out=cand[:], in_=g2ps[64:128, :], func=AF.Tanh)
                # z-path (off critical path, runs while ACT computes cand):
                # zh = (1+y_z)/2;  p = h - zh*h
                zh = spool.tile([64, BS], dt.float32, tag="zh")
                nc.vector.tensor_scalar(zh[:], y[0:64, :], 0.5, 0.5, ALU.mult, ALU.add)
                t1 = spool.tile([64, BS], dt.float32, tag="t1")
                nc.vector.tensor_mul(t1[:], S[0:64, :], zh[:])
                p = spool.tile([64, BS], dt.float32, tag="p")
                nc.vector.tensor_sub(p[:], S[0:64, :], t1[:])
                # h_new = p + zh*cand
                w = spool.tile([64, BS], dt.float32, tag="w")
                nc.vector.tensor_mul(w[:], cand[:], zh[:])
                nc.vector.tensor_add(S2[0:64, :], p[:], w[:])
                # store pre-update states: fwd block s+1, bwd block 62-s
                fb = s + 1
                bb = SEQ - 2 - s
                nc.gpsimd.tensor_copy(HT[fb // 16][0:8, (fb % 16) * BS : (fb % 16) * BS + BS],
                                      S2[0:8, :])
                nc.gpsimd.tensor_copy(HT[bb // 16][32:40, (bb % 16) * BS : (bb % 16) * BS + BS],
                                      S2[32:40, :])
                S = S2

            # ---- phase 3: projection + log_softmax, software-pipelined ----
            proj_order = (1, 2, 0, 3)

            def prep_tile(m):
                nc.vector.tensor_copy(HTb[m][0:8, :], HT[m][0:8, :])
                nc.vector.tensor_copy(HTb[m][32:40, :], HT[m][32:40, :])
                sums = smp.tile([128, NG], dt.float32, tag=f"sums{m}")
                return sums

            def p1_group(m, g, sums):
                lps = prjp.tile([128, 2, 512], dt.float32, tag="lps")
                for h in range(2):
                    j = 2 * g + h
                    nc.tensor.matmul(lps[:, h, 0:NCHUNK], lhsT=HTb[m][:],
                                     rhs=wout_sb[:, j * NCHUNK : (j + 1) * NCHUNK],
                                     start=True, stop=True)
                esc = escp.tile([128, 2, 512], dt.bfloat16, tag="esc")
                nc.scalar.activation(out=esc[:, :, 0:NCHUNK], in_=lps[:, :, 0:NCHUNK],
                                     func=AF.Exp, accum_out=sums[:, g : g + 1])

            def finish_nlz(m, sums):
                # nlz = -ln(sum(exp)) computed on DVE (exponent extraction +
                # ln(1+t) polynomial) so the ACT table set never switches.
                ws = smp.tile([128, 8], dt.float32, tag=f"nlzw{m}")
                wi = smp.tile([128, 2], dt.int32, tag=f"nlzi{m}")
                nc.vector.reduce_sum(out=ws[:, 0:1], in_=sums[:, 0:NG],
                                     axis=mybir.AxisListType.X)
                bits = ws[:, 0:1].bitcast(dt.int32)
                # exponent as float without an int->float cast: (bits>>23)
                # OR 0x4B000000 bitcasts to the float 8388608 + e_raw
                nc.vector.tensor_scalar(wi[:, 0:1], bits, 23, 0x4B000000,
                                        ALU.logical_shift_right, ALU.bitwise_or)
                nc.vector.tensor_scalar_sub(ws[:, 1:2], wi[:, 0:1].bitcast(dt.float32),
                                            8388608.0 + 127.0)  # e = e_raw - 127
                nc.vector.tensor_scalar(wi[:, 1:2], bits, 0x007FFFFF, 0x3F800000,
                                        ALU.bitwise_and, ALU.bitwise_or)
                mf = wi[:, 1:2].bitcast(dt.float32)
                nc.vector.tensor_scalar_sub(ws[:, 2:3], mf, 1.0)        # t = m-1
                t_ = ws[:, 2:3]
                pp = ws[:, 3:4]
                nc.vector.tensor_scalar(pp, t_, _LN_C[0], _LN_C[1], ALU.mult, ALU.add)
                for ck in _LN_C[2:]:
                    nc.vector.tensor_mul(pp, pp, t_)
                    nc.vector.tensor_scalar_add(pp, pp, float(ck))
                # lse = e*ln2 + ln(m);  nlz = -lse
                nc.vector.scalar_tensor_tensor(ws[:, 4:5], ws[:, 1:2], _LN2, pp,
                                               ALU.mult, ALU.add)
                nc.vector.tensor_scalar_mul(ws[:, 5:6], ws[:, 4:5], -1.0)
                return ws[:, 5:6]

            def p2_group(m, g, stg, nlz, drain_act=False):
                lps = prjp.tile([128, 2, 512], dt.float32, tag="lps")
                for h in range(2):
                    j = 2 * g + h
                    nc.tensor.matmul(lps[:, h, 0:NCHUNK], lhsT=HTb[m][:],
                                     rhs=wout_sb[:, j * NCHUNK : (j + 1) * NCHUNK],
                                     start=True, stop=True)
                gg = g % PIECE_G
                dst = stg[:, 2 * gg * NCHUNK : (2 * gg + 2) * NCHUNK]
                dst = dst.rearrange("p (two c) -> p two c", two=2)
                if drain_act:
                    nc.scalar.add(dst, lps[:, :, 0:NCHUNK], nlz)
                else:
                    nc.vector.tensor_scalar_add(dst, lps[:, :, 0:NCHUNK], nlz)

            def p2_piece_dma(m, piece, stg):
                nc.sync.dma_start(
                    out_d[m * 128 : (m + 1) * 128,
                          piece * PIECE_G * 2 * NCHUNK : (piece + 1) * PIECE_G * 2 * NCHUNK],
                    stg[:],
                )

            # phase A: pass 1 of the first tile alone (overlaps the scan tail)
            first = proj_order[0]
            sums_cur = prep_tile(first)
            for g in range(NG):
                p1_group(first, g, sums_cur)
            nlz_prev = finish_nlz(first, sums_cur)

            # middle phases: pass 2 of prev tile + pass 1 of cur tile
            for i in range(1, NT):
                cur, prev = proj_order[i], proj_order[i - 1]
                sums_cur = prep_tile(cur)
                for piece in range(NG // PIECE_G):
                    stg = stgp.tile([128, PIECE_G * 2 * NCHUNK], dt.float32, tag="stg")
                    for gg in range(PIECE_G):
                        g = piece * PIECE_G + gg
                        p2_group(prev, g, stg, nlz_prev, drain_act=(gg % 4 == 3))
                        p1_group(cur, g, sums_cur)
                    p2_piece_dma(prev, piece, stg)
                nlz_prev = finish_nlz(cur, sums_cur)

            # final phase: pass 2 of the last tile alone (split ACT/DVE)
            last = proj_order[NT - 1]
            for piece in range(NG // PIECE_G):
                stg = stgp.tile([128, PIECE_G * 2 * NCHUNK], dt.float32, tag="stg")
                for gg in range(PIECE_G):
                    g = piece * PIECE_G + gg
                    p2_group(last, g, stg, nlz_prev, drain_act=(gg % 2 == 1))
                p2_piece_dma(last, piece, stg)

            for pool in (prjp, g2psp, zrpsp):
                pool.release()

    nc.compile()
    return nc


def _prep_weights(embeddings, Wz1, bz1, Wr1, br1, Wh1, bh1, Wz2, bz2, Wr2, br2, Wh2, bh2,
                  Wout, bout):
    f32 = np.float32
    bf16 = ml_dtypes.bfloat16
    emb = np.ascontiguousarray(np.asarray(embeddings, dtype=f32))
    vocab = emb.shape[0]

    Wz1, Wr1, Wh1 = (np.asarray(a, dtype=f32) for a in (Wz1, Wr1, Wh1))
    Wz2, Wr2, Wh2 = (np.asarray(a, dtype=f32) for a in (Wz2, Wr2, Wh2))

    # We_all [33, 42]: embedding-side weights for all gates, bias row folded
    # in, columns in P20 row layout: 0=z1, 1=r1, 2:10=h1, 32=z2, 33=r2,
    # 34:42=h2.  cat = [h, e].
    wea = np.zeros((EMB + 1, 42), dtype=f32)
    wea[:EMB, 0] = Wz1[HID:, 0]
    wea[:EMB, 1] = Wr1[HID:, 0]
    wea[:EMB, 2:10] = Wh1[HID:, :]
    wea[:EMB, 32] = Wz2[HID:, 0]
    wea[:EMB, 33] = Wr2[HID:, 0]
    wea[:EMB, 34:42] = Wh2[HID:, :]
    wea[EMB, 0] = np.asarray(bz1)[0]
    wea[EMB, 1] = np.asarray(br1)[0]
    wea[EMB, 2:10] = np.asarray(bh1)
    wea[EMB, 32] = np.asarray(bz2)[0]
    wea[EMB, 33] = np.asarray(br2)[0]
    wea[EMB, 34:42] = np.asarray(bh2)

    # Wzr [106, 104] bf16: hidden-side z/r weights (x0.5 for the tanh
    # sigmoid) plus selector rows passing the precomputed input-side terms
    # carried in S rows 64:106.  Out cols replicated 8-wide per gate so the
    # PSUM output is the broadcast gate plane.
    wzr = np.zeros((106, 104), dtype=f32)
    for j in range(HID):
        wzr[0:HID, j] = 0.5 * Wz1[:HID, 0]
        wzr[64, j] = 0.5
        wzr[32 : 32 + HID, 32 + j] = 0.5 * Wz2[:HID, 0]
        wzr[96, 32 + j] = 0.5
        wzr[0:HID, 64 + j] = 0.5 * Wr1[:HID, 0]
        wzr[65, 64 + j] = 0.5
        wzr[32 : 32 + HID, 96 + j] = 0.5 * Wr2[:HID, 0]
        wzr[97, 96 + j] = 0.5

    # Whh [106, 128] bf16: cols 0:8/32:40 = G' = 0.5*(Whh.T h) spread; cols
    # 64:72/96:104 = G' + eh via selector rows; other cols zero.
    whh = np.zeros((106, 128), dtype=f32)
    whh[0:HID, 0:HID] = 0.5 * Wh1[:HID, :]
    whh[32 : 32 + HID, 32 : 32 + HID] = 0.5 * Wh2[:HID, :]
    whh[0:HID, 64 : 64 + HID] = 0.5 * Wh1[:HID, :]
    whh[32 : 32 + HID, 96 : 96 + HID] = 0.5 * Wh2[:HID, :]
    for k in range(HID):
        whh[66 + k, 64 + k] = 1.0
        whh[98 + k, 96 + k] = 1.0

    # Wout_aug [128, vocab] bf16: rows 0:8 fwd-h weights, 32:40 bwd-h
    # weights, 64 = bout; all other rows zero (matching HTb junk lanes).
    # Padded to K=128 so projection matmuls drive the full PE array.
    Wout = np.asarray(Wout, dtype=f32)
    wout_aug = np.zeros((128, vocab), dtype=f32)
    wout_aug[0:HID, :] = Wout[0:HID, :]
    wout_aug[32 : 32 + HID, :] = Wout[HID:, :]
    wout_aug[64, :] = np.asarray(bout, dtype=f32)

    return dict(emb=emb, wea=wea,
                wzr=wzr.astype(bf16), whh=whh.astype(bf16),
                wout=wout_aug.astype(bf16), vocab=vocab)


def run(inputs, trace=False):
    from concourse.bass_utils import run_bass_kernel_spmd

    w = _prep_weights(
        inputs["embeddings"],
        inputs["Wz1"], inputs["bz1"], inputs["Wr1"], inputs["br1"],
        inputs["Wh1"], inputs["bh1"],
        inputs["Wz2"], inputs["bz2"], inputs["Wr2"], inputs["br2"],
        inputs["Wh2"], inputs["bh2"],
        inputs["Wout"], inputs["bout"],
    )
    vocab = w.pop("vocab")
    x = np.ascontiguousarray(np.asarray(inputs["x"], dtype=np.int32))
    assert x.shape == (SEQ, BATCH)

    key = ("module", vocab)
    if key not in _module_cache:
        _module_cache[key] = _build_module(vocab=vocab)
    nc = _module_cache[key]

    in_maps = []
    for c in range(NCORES):
        m = dict(w)
        m["x"] = np.ascontiguousarray(x[:, c * BS : (c + 1) * BS])
        in_maps.append(m)

    res = run_bass_kernel_spmd(nc, in_maps, core_ids=list(range(NCORES)), trace=trace)
    shards = [res.results[c]["out"].reshape(SEQ, BS, vocab) for c in range(NCORES)]
    out = np.concatenate(shards, axis=1)
    return out, res


def kernel(**inputs):
    out, _ = run(inputs)
    return out
